# revision 1
# baseline (speedup 1.0000x reference)
"""Trainium2 Bass kernel for the GNO (Galerkin-type linear attention) model.

Reference computation per batch element b (N=4096 tokens, d=64):
    h = x @ lift_w + lift_b
    for each of 4 layers:
        q = h@q_w+q_b ; k = h@k_w+k_b ; v = h@v_w+v_b
        kern     = (q @ k^T) / sqrt(d)          # [N, N], no softmax!
        integral = (kern @ v) / N               # [N, d]
        h        = gelu(h@blk_w+blk_b + integral)
    out = h @ proj_w + proj_b

Because there is no softmax, (q k^T) v == q (k^T v), so each layer only
needs the tiny [64,64] moment matrix ktv = k^T v.  Further,
    integral = q @ (s*ktv)  = H_aug^T (q_w_aug @ (s*ktv))
    wh + integral           = H_aug^T (blk_w_aug + q_w_aug @ (s*ktv))
with H_aug = [h^T; 1] (a ones row folds every bias into the weights), so
the whole layer update is ONE [65,64] combined weight matmul + gelu.

Sharding: batch is 2 -> data-parallel on 2 NeuronCores, one batch element
per core, everything resident in SBUF.  Sequence-sharding wider would need
a per-layer AllReduce of ktv whose ~10us latency floor exceeds the whole
per-layer compute, so it loses.
"""

import os
import sys

for _p in ("/opt/trn_rl_repo", "/root/.axon_site/_ro/trn_rl_repo"):
    if os.path.isdir(_p) and _p not in sys.path:
        sys.path.append(_p)

import numpy as np

N = 4096          # tokens per batch element (64*64)
D = 64            # hidden
DA = D + 1        # hidden + ones row
L = 4             # layers
B = 2             # batch / cores used
SCALE = (1.0 / np.sqrt(np.float32(D))) / np.float32(N)

_CACHE = {}


def _build_nc():
    """Build + compile the per-core Bass program (identical on both cores)."""
    import concourse.bass as bass
    import concourse.tile as tile
    from concourse import bacc, mybir

    f32 = mybir.dt.float32
    ts = bass.ts
    GELU = mybir.ActivationFunctionType.Gelu

    nc = bacc.Bacc("TRN2", target_bir_lowering=False, debug=False, num_devices=B)

    xt_d = nc.dram_tensor("xt", [4, N], f32, kind="ExternalInput")
    lift_d = nc.dram_tensor("liftw", [4, DA], f32, kind="ExternalInput")
    kvw_d = nc.dram_tensor("kvw", [DA, L * 2 * D], f32, kind="ExternalInput")
    qts_d = nc.dram_tensor("qts", [D, L * DA], f32, kind="ExternalInput")
    blkw_d = nc.dram_tensor("blkw", [DA, L * D], f32, kind="ExternalInput")
    proj_d = nc.dram_tensor("projw", [DA, 1], f32, kind="ExternalInput")
    y_d = nc.dram_tensor("y", [1, N], f32, kind="ExternalOutput")

    PS = bass.MemorySpace.PSUM

    with tile.TileContext(nc) as tc:
        with (
            tc.tile_pool(name="consts", bufs=1) as consts,
            tc.tile_pool(name="hbuf", bufs=1) as hbuf,
            tc.tile_pool(name="kvsb", bufs=3) as kvsb,
            tc.tile_pool(name="small", bufs=2) as small,
            tc.tile_pool(name="ps_kv", bufs=2, space=PS) as ps_kv,
            tc.tile_pool(name="ps_sm", bufs=2, space=PS) as ps_sm,
            tc.tile_pool(name="ps_up", bufs=2, space=PS) as ps_up,
        ):
            # ---- load everything into SBUF -------------------------------
            xt = consts.tile([4, N], f32, tag="xt")
            nc.sync.dma_start(xt[:], xt_d.ap())
            liftw = consts.tile([4, DA], f32, tag="liftw")
            nc.sync.dma_start(liftw[:], lift_d.ap())
            kvw = consts.tile([DA, L * 2 * D], f32, tag="kvw")
            nc.sync.dma_start(kvw[:], kvw_d.ap())
            qts = consts.tile([D, L * DA], f32, tag="qts")
            nc.sync.dma_start(qts[:], qts_d.ap())
            blkw = consts.tile([DA, L * D], f32, tag="blkw")
            nc.sync.dma_start(blkw[:], blkw_d.ap())
            projw = consts.tile([DA, 1], f32, tag="projw")
            nc.sync.dma_start(projw[:], proj_d.ap())

            # two ping-pong H_aug buffers, [65, 4096] each
            H0 = hbuf.tile([DA, N], f32, tag="h0")
            H1 = hbuf.tile([DA, N], f32, tag="h1")
            # H1's ones row never gets written by the layer loop; seed it
            # from the ones row of x^T_aug.  H0's comes from the lift matmul.
            nc.sync.dma_start(H1[D : D + 1, :], xt_d.ap()[3:4, :])

            # ---- lift: H0 = lift_aug^T @ xt  ([65, 4096]) ----------------
            for c in range(8):
                ps = ps_up.tile([DA, 512], f32, tag="up")
                nc.tensor.matmul(ps[:], liftw[:], xt[:, ts(c, 512)],
                                 start=True, stop=True)
                nc.vector.tensor_copy(H0[:, ts(c, 512)], ps[:])

            # ---- layers --------------------------------------------------
            for l in range(L):
                cur = H0 if l % 2 == 0 else H1
                nxt = H1 if l % 2 == 0 else H0
                kvw_l = kvw[:, l * 2 * D : (l + 1) * 2 * D]

                ktv_ps = ps_sm.tile([D, D], f32, tag="sm")
                for j in range(8):
                    kv_ps = ps_kv.tile([128, 512], f32, tag="kv")
                    for k in range(4):
                        t = 4 * j + k
                        # KV_nat tile [128, 128] = H_chunk^T @ kvw_aug
                        nc.tensor.matmul(kv_ps[:, ts(k, 128)],
                                         cur[:, ts(t, 128)], kvw_l,
                                         start=True, stop=True)
                    kv_sb = kvsb.tile([128, 512], f32, tag="kvt")
                    nc.vector.tensor_copy(kv_sb[:], kv_ps[:])
                    for k in range(4):
                        first = (j == 0 and k == 0)
                        last = (j == 7 and k == 3)
                        # ktv += K_tile^T @ V_tile
                        nc.tensor.matmul(ktv_ps[:],
                                         kv_sb[:, k * 128 : k * 128 + 64],
                                         kv_sb[:, k * 128 + 64 : k * 128 + 128],
                                         start=first, stop=last)

                ktv_sb = small.tile([D, D], f32, tag="ktv")
                nc.vector.tensor_copy(ktv_sb[:], ktv_ps[:])

                # W_upd = blk_w_aug + q_w_aug*s @ ktv   ([65, 64])
                weff_ps = ps_sm.tile([DA, D], f32, tag="sm")
                nc.tensor.matmul(weff_ps[:], qts[:, l * DA : (l + 1) * DA],
                                 ktv_sb[:], start=True, stop=True)
                wupd_sb = small.tile([DA, D], f32, tag="wupd")
                nc.vector.tensor_add(wupd_sb[:], weff_ps[:],
                                     blkw[:, l * D : (l + 1) * D])

                # h' = gelu(H_aug^T @ W_upd), written transposed into nxt
                for c in range(4):
                    up_ps = ps_up.tile([D, 1024], f32, tag="up")
                    for i in range(2):
                        nc.tensor.matmul(
                            up_ps[:, ts(i, 512)], wupd_sb[:],
                            cur[:, 1024 * c + 512 * i : 1024 * c + 512 * (i + 1)],
                            start=True, stop=True)
                    nc.scalar.activation(nxt[0:D, ts(c, 1024)], up_ps[:], GELU)

            # ---- proj: y = proj_aug^T @ H_final  ([1, 4096]) -------------
            Hf = H0 if L % 2 == 0 else H1
            out_sb = consts.tile([1, N], f32, tag="out")
            for c in range(8):
                pr_ps = ps_sm.tile([1, 512], f32, tag="sm")
                nc.tensor.matmul(pr_ps[:], projw[:], Hf[:, ts(c, 512)],
                                 start=True, stop=True)
                nc.vector.tensor_copy(out_sb[0:1, ts(c, 512)], pr_ps[:])
            nc.sync.dma_start(y_d.ap(), out_sb[:])

    nc.compile()
    return nc


def _prep_inputs(x, lift_w, lift_b, blk_w, blk_b, q_w, q_b, k_w, k_b, v_w,
                 v_b, proj_w, proj_b):
    """Host-side weight packing (tiny [64,64] reshuffles, negligible cost)."""
    f = lambda a: np.asarray(a, dtype=np.float32)
    x = f(x)
    lift_w, lift_b = f(lift_w), f(lift_b)
    blk_w, blk_b = f(blk_w), f(blk_b)
    q_w, q_b, k_w, k_b, v_w, v_b = f(q_w), f(q_b), f(k_w), f(k_b), f(v_w), f(v_b)
    proj_w, proj_b = f(proj_w), f(proj_b)

    lift_aug = np.zeros((4, DA), np.float32)
    lift_aug[:3, :D] = lift_w
    lift_aug[3, :D] = lift_b
    lift_aug[3, D] = 1.0  # makes the lift matmul emit H0's ones row

    kvw = np.concatenate(
        [np.concatenate([np.vstack([k_w[l], k_b[l][None]]),
                         np.vstack([v_w[l], v_b[l][None]])], axis=1)
         for l in range(L)], axis=1).astype(np.float32)          # [65, 512]
    qts = np.concatenate(
        [(np.vstack([q_w[l], q_b[l][None]]) * SCALE).T
         for l in range(L)], axis=1).astype(np.float32)          # [64, 260]
    blkw = np.concatenate(
        [np.vstack([blk_w[l], blk_b[l][None]]) for l in range(L)],
        axis=1).astype(np.float32)                               # [65, 256]
    proj = np.vstack([proj_w, proj_b[None]]).astype(np.float32)  # [65, 1]

    in_maps = []
    for b in range(B):
        xt = np.concatenate([x[b].reshape(N, 3).T,
                             np.ones((1, N), np.float32)], axis=0)
        in_maps.append({"xt": np.ascontiguousarray(xt), "liftw": lift_aug,
                        "kvw": kvw, "qts": qts, "blkw": blkw, "projw": proj})
    return in_maps, x.shape


def _get_runner():
    """Compile once, return a fn(in_maps) -> list[{name: np.ndarray}]."""
    if "runner" in _CACHE:
        return _CACHE["runner"]

    import jax
    from jax.sharding import Mesh, PartitionSpec
    try:
        from jax.experimental.shard_map import shard_map
    except ImportError:  # newer jax
        from jax.sharding import shard_map
    from concourse import mybir
    from concourse.bass2jax import (_bass_exec_p, install_neuronx_cc_hook,
                                    partition_id_tensor)

    nc = _build_nc()
    install_neuronx_cc_hook()

    partition_name = (nc.partition_id_tensor.name
                      if nc.partition_id_tensor else None)
    in_names, out_names, out_avals, zero_outs = [], [], [], []
    for alloc in nc.m.functions[0].allocations:
        if not isinstance(alloc, mybir.MemoryLocationSet):
            continue
        name = alloc.memorylocations[0].name
        if alloc.kind == "ExternalInput":
            if name != partition_name:
                in_names.append(name)
        elif alloc.kind == "ExternalOutput":
            shape = tuple(alloc.tensor_shape)
            dtype = mybir.dt.np(alloc.dtype)
            out_names.append(name)
            out_avals.append(jax.core.ShapedArray(shape, dtype))
            zero_outs.append(np.zeros(shape, dtype))
    n_params = len(in_names)
    n_outs = len(out_avals)
    all_in_names = in_names + out_names + ([partition_name] if partition_name else [])
    donate = tuple(range(n_params, n_params + n_outs))

    def _body(*args):
        operands = list(args)
        if partition_name is not None:
            operands.append(partition_id_tensor())
        return tuple(_bass_exec_p.bind(
            *operands, out_avals=tuple(out_avals), in_names=tuple(all_in_names),
            out_names=tuple(out_names), lowering_input_output_aliases=(),
            sim_require_finite=True, sim_require_nnan=True, nc=nc))

    devices = jax.devices()[:B]
    mesh = Mesh(np.asarray(devices), ("core",))
    sharded = jax.jit(
        shard_map(_body, mesh=mesh,
                  in_specs=(PartitionSpec("core"),) * (n_params + n_outs),
                  out_specs=(PartitionSpec("core"),) * n_outs,
                  check_rep=False),
        donate_argnums=donate, keep_unused=True)

    def run(in_maps):
        per_core = [[np.asarray(m[name]) for name in in_names] for m in in_maps]
        concat_in = [np.concatenate([per_core[c][i] for c in range(B)], axis=0)
                     for i in range(n_params)]
        big_zeros = [np.concatenate([z] * B, axis=0) for z in zero_outs]
        outs = jax.block_until_ready(sharded(*concat_in, *big_zeros))
        results = []
        for c in range(B):
            r = {}
            for i, name in enumerate(out_names):
                rows = out_avals[i].shape[0]
                r[name] = np.asarray(outs[i][c * rows : (c + 1) * rows])
            results.append(r)
        return results

    _CACHE["runner"] = run
    return run


def kernel(**inputs) -> np.ndarray:
    in_maps, x_shape = _prep_inputs(**inputs)
    run = _get_runner()
    results = run(in_maps)
    out = np.stack([results[b]["y"].reshape(x_shape[1], x_shape[2], 1)
                    for b in range(B)])
    return out.astype(np.float32)



# revision 3
# speedup vs baseline: 2.5969x; 2.5969x over previous
"""Trainium2 Bass kernel for the GNO (Galerkin-type linear attention) model.

Reference computation per batch element b (N=4096 tokens, d=64):
    h = x @ lift_w + lift_b
    for each of 4 layers:
        q = h@q_w+q_b ; k = h@k_w+k_b ; v = h@v_w+v_b
        kern     = (q @ k^T) / sqrt(d)          # [N, N], no softmax!
        integral = (kern @ v) / N               # [N, d]
        h        = gelu(h@blk_w+blk_b + integral)
    out = h @ proj_w + proj_b

Because there is no softmax, (q k^T) v == q (k^T v), so each layer only
needs the tiny [64,64] moment matrix ktv = k^T v.  Further,
    integral = q @ (s*ktv)  = H_aug^T (q_w_aug @ (s*ktv))
    wh + integral           = H_aug^T (blk_w_aug + q_w_aug @ (s*ktv))
with H_aug = [h^T; 1] (a ones row folds every bias into the weights), so
the whole layer update is ONE [65,64] combined weight matmul + gelu.

Sharding: batch is 2 -> data-parallel on 2 NeuronCores, one batch element
per core, everything resident in SBUF.  Sequence-sharding wider would need
a per-layer AllReduce of ktv whose ~10us latency floor exceeds the whole
per-layer compute, so it loses.
"""

import os
import sys

for _p in ("/opt/trn_rl_repo", "/root/.axon_site/_ro/trn_rl_repo"):
    if os.path.isdir(_p) and _p not in sys.path:
        sys.path.append(_p)

import numpy as np

N = 4096          # tokens per batch element (64*64)
D = 64            # hidden
DA = D + 1        # hidden + ones row
L = 4             # layers
B = 2             # batch / cores used
SCALE = (1.0 / np.sqrt(np.float32(D))) / np.float32(N)

_CACHE = {}


def _build_nc():
    """Build + compile the per-core Bass program (identical on both cores).

    All large matmuls run in fp16 (1 PE cycle/row vs fp32's 4): the PE is
    row-throughput-bound here, so this is a ~4x tensor-time cut.  PSUM
    accumulation stays fp32, and the tiny ktv -> W_upd chain stays fp32,
    so the only precision loss is fp16 rounding (~5e-4) of H / weights.
    """
    import concourse.bass as bass
    import concourse.tile as tile
    from concourse import bacc, mybir

    f32 = mybir.dt.float32
    f16 = mybir.dt.float16
    ts = bass.ts
    GELU = mybir.ActivationFunctionType.Gelu

    nc = bacc.Bacc("TRN2", target_bir_lowering=False, debug=False, num_devices=B)

    xt_d = nc.dram_tensor("xt", [4, N], f16, kind="ExternalInput")
    lift_d = nc.dram_tensor("liftw", [4, DA], f16, kind="ExternalInput")
    kvw_d = nc.dram_tensor("kvw", [DA, L * 2 * D], f16, kind="ExternalInput")
    qts_d = nc.dram_tensor("qts", [D, L * DA], f32, kind="ExternalInput")
    blkw_d = nc.dram_tensor("blkw", [DA, L * D], f32, kind="ExternalInput")
    proj_d = nc.dram_tensor("projw", [DA, 1], f16, kind="ExternalInput")
    y_d = nc.dram_tensor("y", [1, N], f32, kind="ExternalOutput")

    PS = bass.MemorySpace.PSUM

    with tile.TileContext(nc) as tc:
        with (
            tc.tile_pool(name="consts", bufs=1) as consts,
            tc.tile_pool(name="hbuf", bufs=1) as hbuf,
            tc.tile_pool(name="kvsb", bufs=3) as kvsb,
            tc.tile_pool(name="small", bufs=2) as small,
            tc.tile_pool(name="ps_kv", bufs=2, space=PS) as ps_kv,
            tc.tile_pool(name="ps_sm", bufs=2, space=PS) as ps_sm,
            tc.tile_pool(name="ps_up", bufs=2, space=PS) as ps_up,
        ):
            # ---- load everything into SBUF -------------------------------
            xt = consts.tile([4, N], f16, tag="xt")
            nc.sync.dma_start(xt[:], xt_d.ap())
            liftw = consts.tile([4, DA], f16, tag="liftw")
            nc.sync.dma_start(liftw[:], lift_d.ap())
            kvw = consts.tile([DA, L * 2 * D], f16, tag="kvw")
            nc.sync.dma_start(kvw[:], kvw_d.ap())
            qts = consts.tile([D, L * DA], f32, tag="qts")
            nc.sync.dma_start(qts[:], qts_d.ap())
            blkw = consts.tile([DA, L * D], f32, tag="blkw")
            nc.sync.dma_start(blkw[:], blkw_d.ap())
            projw = consts.tile([DA, 1], f16, tag="projw")
            nc.sync.dma_start(projw[:], proj_d.ap())

            # two ping-pong H_aug buffers, [65, 4096] each
            H0 = hbuf.tile([DA, N], f16, tag="h0")
            H1 = hbuf.tile([DA, N], f16, tag="h1")
            # H1's ones row never gets written by the layer loop; seed it
            # from the ones row of x^T_aug.  H0's comes from the lift matmul.
            nc.sync.dma_start(H1[D : D + 1, :], xt_d.ap()[3:4, :])

            # ---- lift: H0 = lift_aug^T @ xt  ([65, 4096]) ----------------
            for c in range(8):
                ps = ps_up.tile([DA, 512], f32, tag="up")
                nc.tensor.matmul(ps[:], liftw[:], xt[:, ts(c, 512)],
                                 start=True, stop=True)
                nc.vector.tensor_copy(H0[:, ts(c, 512)], ps[:])

            # ---- layers --------------------------------------------------
            for l in range(L):
                cur = H0 if l % 2 == 0 else H1
                nxt = H1 if l % 2 == 0 else H0
                kvw_l = kvw[:, l * 2 * D : (l + 1) * 2 * D]

                ktv_ps = ps_sm.tile([D, D], f32, tag="sm")
                for j in range(8):
                    kv_ps = ps_kv.tile([128, 512], f32, tag="kv")
                    for k in range(4):
                        t = 4 * j + k
                        # KV_nat tile [128, 128] = H_chunk^T @ kvw_aug
                        nc.tensor.matmul(kv_ps[:, ts(k, 128)],
                                         cur[:, ts(t, 128)], kvw_l,
                                         start=True, stop=True)
                    kv_sb = kvsb.tile([128, 512], f16, tag="kvt")
                    nc.vector.tensor_copy(kv_sb[:], kv_ps[:])
                    for k in range(4):
                        first = (j == 0 and k == 0)
                        last = (j == 7 and k == 3)
                        # ktv += K_tile^T @ V_tile
                        nc.tensor.matmul(ktv_ps[:],
                                         kv_sb[:, k * 128 : k * 128 + 64],
                                         kv_sb[:, k * 128 + 64 : k * 128 + 128],
                                         start=first, stop=last)

                ktv_sb = small.tile([D, D], f32, tag="ktv")
                nc.vector.tensor_copy(ktv_sb[:], ktv_ps[:])

                # W_upd = blk_w_aug + q_w_aug*s @ ktv   ([65, 64])
                weff_ps = ps_sm.tile([DA, D], f32, tag="sm")
                nc.tensor.matmul(weff_ps[:], qts[:, l * DA : (l + 1) * DA],
                                 ktv_sb[:], start=True, stop=True)
                wupd_sb = small.tile([DA, D], f16, tag="wupd")
                nc.vector.tensor_add(wupd_sb[:], weff_ps[:],
                                     blkw[:, l * D : (l + 1) * D])

                # h' = gelu(H_aug^T @ W_upd), written transposed into nxt
                for c in range(4):
                    up_ps = ps_up.tile([D, 1024], f32, tag="up")
                    for i in range(2):
                        nc.tensor.matmul(
                            up_ps[:, ts(i, 512)], wupd_sb[:],
                            cur[:, 1024 * c + 512 * i : 1024 * c + 512 * (i + 1)],
                            start=True, stop=True)
                    nc.scalar.activation(nxt[0:D, ts(c, 1024)], up_ps[:], GELU)

            # ---- proj: y = proj_aug^T @ H_final  ([1, 4096]) -------------
            Hf = H0 if L % 2 == 0 else H1
            out_sb = consts.tile([1, N], f32, tag="out")
            for c in range(8):
                pr_ps = ps_sm.tile([1, 512], f32, tag="sm")
                nc.tensor.matmul(pr_ps[:], projw[:], Hf[:, ts(c, 512)],
                                 start=True, stop=True)
                nc.vector.tensor_copy(out_sb[0:1, ts(c, 512)], pr_ps[:])
            nc.sync.dma_start(y_d.ap(), out_sb[:])

    nc.compile()
    return nc


def _prep_inputs(x, lift_w, lift_b, blk_w, blk_b, q_w, q_b, k_w, k_b, v_w,
                 v_b, proj_w, proj_b):
    """Host-side weight packing (tiny [64,64] reshuffles, negligible cost)."""
    f = lambda a: np.asarray(a, dtype=np.float32)
    x = f(x)
    lift_w, lift_b = f(lift_w), f(lift_b)
    blk_w, blk_b = f(blk_w), f(blk_b)
    q_w, q_b, k_w, k_b, v_w, v_b = f(q_w), f(q_b), f(k_w), f(k_b), f(v_w), f(v_b)
    proj_w, proj_b = f(proj_w), f(proj_b)

    lift_aug = np.zeros((4, DA), np.float32)
    lift_aug[:3, :D] = lift_w
    lift_aug[3, :D] = lift_b
    lift_aug[3, D] = 1.0  # makes the lift matmul emit H0's ones row

    kvw = np.concatenate(
        [np.concatenate([np.vstack([k_w[l], k_b[l][None]]),
                         np.vstack([v_w[l], v_b[l][None]])], axis=1)
         for l in range(L)], axis=1).astype(np.float16)          # [65, 512]
    qts = np.concatenate(
        [(np.vstack([q_w[l], q_b[l][None]]) * SCALE).T
         for l in range(L)], axis=1).astype(np.float32)          # [64, 260]
    blkw = np.concatenate(
        [np.vstack([blk_w[l], blk_b[l][None]]) for l in range(L)],
        axis=1).astype(np.float32)                               # [65, 256]
    proj = np.vstack([proj_w, proj_b[None]]).astype(np.float16)  # [65, 1]
    lift_aug = lift_aug.astype(np.float16)

    in_maps = []
    for b in range(B):
        xt = np.concatenate([x[b].reshape(N, 3).T,
                             np.ones((1, N), np.float32)], axis=0)
        in_maps.append({"xt": np.ascontiguousarray(xt.astype(np.float16)),
                        "liftw": lift_aug,
                        "kvw": kvw, "qts": qts, "blkw": blkw, "projw": proj})
    return in_maps, x.shape


def _get_runner():
    """Compile once, return a fn(in_maps) -> list[{name: np.ndarray}]."""
    if "runner" in _CACHE:
        return _CACHE["runner"]

    import jax
    from jax.sharding import Mesh, PartitionSpec
    try:
        from jax.experimental.shard_map import shard_map
    except ImportError:  # newer jax
        from jax.sharding import shard_map
    from concourse import mybir
    from concourse.bass2jax import (_bass_exec_p, install_neuronx_cc_hook,
                                    partition_id_tensor)

    nc = _build_nc()
    install_neuronx_cc_hook()

    partition_name = (nc.partition_id_tensor.name
                      if nc.partition_id_tensor else None)
    in_names, out_names, out_avals, zero_outs = [], [], [], []
    for alloc in nc.m.functions[0].allocations:
        if not isinstance(alloc, mybir.MemoryLocationSet):
            continue
        name = alloc.memorylocations[0].name
        if alloc.kind == "ExternalInput":
            if name != partition_name:
                in_names.append(name)
        elif alloc.kind == "ExternalOutput":
            shape = tuple(alloc.tensor_shape)
            dtype = mybir.dt.np(alloc.dtype)
            out_names.append(name)
            out_avals.append(jax.core.ShapedArray(shape, dtype))
            zero_outs.append(np.zeros(shape, dtype))
    n_params = len(in_names)
    n_outs = len(out_avals)
    all_in_names = in_names + out_names + ([partition_name] if partition_name else [])
    donate = tuple(range(n_params, n_params + n_outs))

    def _body(*args):
        operands = list(args)
        if partition_name is not None:
            operands.append(partition_id_tensor())
        return tuple(_bass_exec_p.bind(
            *operands, out_avals=tuple(out_avals), in_names=tuple(all_in_names),
            out_names=tuple(out_names), lowering_input_output_aliases=(),
            sim_require_finite=True, sim_require_nnan=True, nc=nc))

    devices = jax.devices()[:B]
    mesh = Mesh(np.asarray(devices), ("core",))
    sharded = jax.jit(
        shard_map(_body, mesh=mesh,
                  in_specs=(PartitionSpec("core"),) * (n_params + n_outs),
                  out_specs=(PartitionSpec("core"),) * n_outs,
                  check_rep=False),
        donate_argnums=donate, keep_unused=True)

    def run(in_maps):
        per_core = [[np.asarray(m[name]) for name in in_names] for m in in_maps]
        concat_in = [np.concatenate([per_core[c][i] for c in range(B)], axis=0)
                     for i in range(n_params)]
        big_zeros = [np.concatenate([z] * B, axis=0) for z in zero_outs]
        outs = jax.block_until_ready(sharded(*concat_in, *big_zeros))
        results = []
        for c in range(B):
            r = {}
            for i, name in enumerate(out_names):
                rows = out_avals[i].shape[0]
                r[name] = np.asarray(outs[i][c * rows : (c + 1) * rows])
            results.append(r)
        return results

    _CACHE["runner"] = run
    return run


def kernel(**inputs) -> np.ndarray:
    in_maps, x_shape = _prep_inputs(**inputs)
    run = _get_runner()
    results = run(in_maps)
    out = np.stack([results[b]["y"].reshape(x_shape[1], x_shape[2], 1)
                    for b in range(B)])
    return out.astype(np.float32)



# revision 5
# speedup vs baseline: 2.6868x; 1.0346x over previous
"""Trainium2 Bass kernel for the GNO (Galerkin-type linear attention) model.

Reference computation per batch element b (N=4096 tokens, d=64):
    h = x @ lift_w + lift_b
    for each of 4 layers:
        q = h@q_w+q_b ; k = h@k_w+k_b ; v = h@v_w+v_b
        kern     = (q @ k^T) / sqrt(d)          # [N, N], no softmax!
        integral = (kern @ v) / N               # [N, d]
        h        = gelu(h@blk_w+blk_b + integral)
    out = h @ proj_w + proj_b

Because there is no softmax, (q k^T) v == q (k^T v), so each layer only
needs the tiny [64,64] moment matrix ktv = k^T v.  Further,
    integral = q @ (s*ktv)  = H_aug^T (q_w_aug @ (s*ktv))
    wh + integral           = H_aug^T (blk_w_aug + q_w_aug @ (s*ktv))
with H_aug = [h^T; 1] (a ones row folds every bias into the weights), so
the whole layer update is ONE [65,64] combined weight matmul + gelu.

Sharding: batch is 2 -> data-parallel on 2 NeuronCores, one batch element
per core, everything resident in SBUF.  Sequence-sharding wider would need
a per-layer AllReduce of ktv whose ~10us latency floor exceeds the whole
per-layer compute, so it loses.
"""

import os
import sys

for _p in ("/opt/trn_rl_repo", "/root/.axon_site/_ro/trn_rl_repo"):
    if os.path.isdir(_p) and _p not in sys.path:
        sys.path.append(_p)

import numpy as np

N = 4096          # tokens per batch element (64*64)
D = 64            # hidden
DA = D + 1        # hidden + ones row
L = 4             # layers
B = 2             # batch / cores used
SCALE = (1.0 / np.sqrt(np.float32(D))) / np.float32(N)

_CACHE = {}


def _build_nc():
    """Build + compile the per-core Bass program (identical on both cores).

    All large matmuls run in fp16 (1 PE cycle/row vs fp32's 4): the PE is
    row-throughput-bound here, so this is a ~4x tensor-time cut.  PSUM
    accumulation stays fp32, and the tiny ktv -> W_upd chain stays fp32,
    so the only precision loss is fp16 rounding (~5e-4) of H / weights.
    """
    import concourse.bass as bass
    import concourse.tile as tile
    from concourse import bacc, mybir

    f32 = mybir.dt.float32
    f16 = mybir.dt.float16
    ts = bass.ts
    GELU = mybir.ActivationFunctionType.Gelu

    nc = bacc.Bacc("TRN2", target_bir_lowering=False, debug=False, num_devices=B)

    xt_d = nc.dram_tensor("xt", [4, N], f16, kind="ExternalInput")
    lift_d = nc.dram_tensor("liftw", [4, DA], f16, kind="ExternalInput")
    kvw_d = nc.dram_tensor("kvw", [DA, L * 2 * D], f16, kind="ExternalInput")
    qts_d = nc.dram_tensor("qts", [D, L * DA], f32, kind="ExternalInput")
    blkw_d = nc.dram_tensor("blkw", [DA, L * D], f32, kind="ExternalInput")
    proj_d = nc.dram_tensor("projw", [DA, 1], f16, kind="ExternalInput")
    y_d = nc.dram_tensor("y", [1, N], f32, kind="ExternalOutput")

    PS = bass.MemorySpace.PSUM

    with tile.TileContext(nc) as tc:
        with (
            tc.tile_pool(name="consts", bufs=1) as consts,
            tc.tile_pool(name="hbuf", bufs=1) as hbuf,
            tc.tile_pool(name="kvsb", bufs=3) as kvsb,
            tc.tile_pool(name="small", bufs=2) as small,
            tc.tile_pool(name="ps_kv", bufs=3, space=PS) as ps_kv,
            tc.tile_pool(name="ps_sm", bufs=2, space=PS) as ps_sm,
            tc.tile_pool(name="ps_up", bufs=3, space=PS) as ps_up,
        ):
            # ---- load everything into SBUF -------------------------------
            xt = consts.tile([4, N], f16, tag="xt")
            nc.sync.dma_start(xt[:], xt_d.ap())
            liftw = consts.tile([4, DA], f16, tag="liftw")
            nc.sync.dma_start(liftw[:], lift_d.ap())
            kvw = consts.tile([DA, L * 2 * D], f16, tag="kvw")
            nc.sync.dma_start(kvw[:], kvw_d.ap())
            qts = consts.tile([D, L * DA], f32, tag="qts")
            nc.sync.dma_start(qts[:], qts_d.ap())
            blkw = consts.tile([DA, L * D], f32, tag="blkw")
            nc.sync.dma_start(blkw[:], blkw_d.ap())
            projw = consts.tile([DA, 1], f16, tag="projw")
            nc.sync.dma_start(projw[:], proj_d.ap())

            # two ping-pong H_aug buffers, [65, 4096] each
            H0 = hbuf.tile([DA, N], f16, tag="h0")
            H1 = hbuf.tile([DA, N], f16, tag="h1")
            # H1's ones row never gets written by the layer loop; seed it
            # from the ones row of x^T_aug.  H0's comes from the lift matmul.
            nc.sync.dma_start(H1[D : D + 1, :], xt_d.ap()[3:4, :])

            # ---- lift: H0 = lift_aug^T @ xt  ([65, 4096]) ----------------
            for c in range(8):
                ps = ps_up.tile([DA, 512], f32, tag="up")
                nc.tensor.matmul(ps[:], liftw[:], xt[:, ts(c, 512)],
                                 start=True, stop=True)
                nc.vector.tensor_copy(H0[:, ts(c, 512)], ps[:])

            # ---- layers --------------------------------------------------
            for l in range(L):
                cur = H0 if l % 2 == 0 else H1
                nxt = H1 if l % 2 == 0 else H0
                kvw_l = kvw[:, l * 2 * D : (l + 1) * 2 * D]

                # Software pipeline: ktv matmuls run 2 groups behind the KV
                # matmuls so the PE never stalls on the DVE's PSUM->fp16
                # cast of kv_sb.
                ktv_ps = ps_sm.tile([D, D], f32, tag="sm")
                kv_tiles = [None] * 8

                def emit_kv(j):
                    kv_ps = ps_kv.tile([128, 512], f32, tag="kv")
                    for k in range(4):
                        t = 4 * j + k
                        # KV_nat tile [128, 128] = H_chunk^T @ kvw_aug
                        nc.tensor.matmul(kv_ps[:, ts(k, 128)],
                                         cur[:, ts(t, 128)], kvw_l,
                                         start=True, stop=True)
                    kv_sb = kvsb.tile([128, 512], f16, tag="kvt")
                    nc.vector.tensor_copy(kv_sb[:], kv_ps[:])
                    kv_tiles[j] = kv_sb

                def emit_ktv(j):
                    kv_sb = kv_tiles[j]
                    for k in range(4):
                        first = (j == 0 and k == 0)
                        last = (j == 7 and k == 3)
                        # ktv += K_tile^T @ V_tile
                        nc.tensor.matmul(ktv_ps[:],
                                         kv_sb[:, k * 128 : k * 128 + 64],
                                         kv_sb[:, k * 128 + 64 : k * 128 + 128],
                                         start=first, stop=last)

                for j in range(8):
                    emit_kv(j)
                    if j >= 2:
                        emit_ktv(j - 2)
                emit_ktv(6)
                emit_ktv(7)

                ktv_sb = small.tile([D, D], f32, tag="ktv")
                nc.vector.tensor_copy(ktv_sb[:], ktv_ps[:])

                # W_upd = blk_w_aug + q_w_aug*s @ ktv   ([65, 64])
                weff_ps = ps_sm.tile([DA, D], f32, tag="sm")
                nc.tensor.matmul(weff_ps[:], qts[:, l * DA : (l + 1) * DA],
                                 ktv_sb[:], start=True, stop=True)
                wupd_sb = small.tile([DA, D], f16, tag="wupd")
                nc.vector.tensor_add(wupd_sb[:], weff_ps[:],
                                     blkw[:, l * D : (l + 1) * D])

                # h' = gelu(H_aug^T @ W_upd), written transposed into nxt
                for c in range(8):
                    up_ps = ps_up.tile([D, 512], f32, tag="up")
                    nc.tensor.matmul(up_ps[:], wupd_sb[:],
                                     cur[:, ts(c, 512)],
                                     start=True, stop=True)
                    nc.scalar.activation(nxt[0:D, ts(c, 512)], up_ps[:], GELU)

            # ---- proj: y = proj_aug^T @ H_final  ([1, 4096]) -------------
            Hf = H0 if L % 2 == 0 else H1
            out_sb = consts.tile([1, N], f32, tag="out")
            for c in range(8):
                pr_ps = ps_sm.tile([1, 512], f32, tag="sm")
                nc.tensor.matmul(pr_ps[:], projw[:], Hf[:, ts(c, 512)],
                                 start=True, stop=True)
                nc.vector.tensor_copy(out_sb[0:1, ts(c, 512)], pr_ps[:])
            nc.sync.dma_start(y_d.ap(), out_sb[:])

    nc.compile()
    return nc


def _prep_inputs(x, lift_w, lift_b, blk_w, blk_b, q_w, q_b, k_w, k_b, v_w,
                 v_b, proj_w, proj_b):
    """Host-side weight packing (tiny [64,64] reshuffles, negligible cost)."""
    f = lambda a: np.asarray(a, dtype=np.float32)
    x = f(x)
    lift_w, lift_b = f(lift_w), f(lift_b)
    blk_w, blk_b = f(blk_w), f(blk_b)
    q_w, q_b, k_w, k_b, v_w, v_b = f(q_w), f(q_b), f(k_w), f(k_b), f(v_w), f(v_b)
    proj_w, proj_b = f(proj_w), f(proj_b)

    lift_aug = np.zeros((4, DA), np.float32)
    lift_aug[:3, :D] = lift_w
    lift_aug[3, :D] = lift_b
    lift_aug[3, D] = 1.0  # makes the lift matmul emit H0's ones row

    kvw = np.concatenate(
        [np.concatenate([np.vstack([k_w[l], k_b[l][None]]),
                         np.vstack([v_w[l], v_b[l][None]])], axis=1)
         for l in range(L)], axis=1).astype(np.float16)          # [65, 512]
    qts = np.concatenate(
        [(np.vstack([q_w[l], q_b[l][None]]) * SCALE).T
         for l in range(L)], axis=1).astype(np.float32)          # [64, 260]
    blkw = np.concatenate(
        [np.vstack([blk_w[l], blk_b[l][None]]) for l in range(L)],
        axis=1).astype(np.float32)                               # [65, 256]
    proj = np.vstack([proj_w, proj_b[None]]).astype(np.float16)  # [65, 1]
    lift_aug = lift_aug.astype(np.float16)

    in_maps = []
    for b in range(B):
        xt = np.concatenate([x[b].reshape(N, 3).T,
                             np.ones((1, N), np.float32)], axis=0)
        in_maps.append({"xt": np.ascontiguousarray(xt.astype(np.float16)),
                        "liftw": lift_aug,
                        "kvw": kvw, "qts": qts, "blkw": blkw, "projw": proj})
    return in_maps, x.shape


def _get_runner():
    """Compile once, return a fn(in_maps) -> list[{name: np.ndarray}]."""
    if "runner" in _CACHE:
        return _CACHE["runner"]

    import jax
    from jax.sharding import Mesh, PartitionSpec
    try:
        from jax.experimental.shard_map import shard_map
    except ImportError:  # newer jax
        from jax.sharding import shard_map
    from concourse import mybir
    from concourse.bass2jax import (_bass_exec_p, install_neuronx_cc_hook,
                                    partition_id_tensor)

    nc = _build_nc()
    install_neuronx_cc_hook()

    partition_name = (nc.partition_id_tensor.name
                      if nc.partition_id_tensor else None)
    in_names, out_names, out_avals, zero_outs = [], [], [], []
    for alloc in nc.m.functions[0].allocations:
        if not isinstance(alloc, mybir.MemoryLocationSet):
            continue
        name = alloc.memorylocations[0].name
        if alloc.kind == "ExternalInput":
            if name != partition_name:
                in_names.append(name)
        elif alloc.kind == "ExternalOutput":
            shape = tuple(alloc.tensor_shape)
            dtype = mybir.dt.np(alloc.dtype)
            out_names.append(name)
            out_avals.append(jax.core.ShapedArray(shape, dtype))
            zero_outs.append(np.zeros(shape, dtype))
    n_params = len(in_names)
    n_outs = len(out_avals)
    all_in_names = in_names + out_names + ([partition_name] if partition_name else [])
    donate = tuple(range(n_params, n_params + n_outs))

    def _body(*args):
        operands = list(args)
        if partition_name is not None:
            operands.append(partition_id_tensor())
        return tuple(_bass_exec_p.bind(
            *operands, out_avals=tuple(out_avals), in_names=tuple(all_in_names),
            out_names=tuple(out_names), lowering_input_output_aliases=(),
            sim_require_finite=True, sim_require_nnan=True, nc=nc))

    devices = jax.devices()[:B]
    mesh = Mesh(np.asarray(devices), ("core",))
    sharded = jax.jit(
        shard_map(_body, mesh=mesh,
                  in_specs=(PartitionSpec("core"),) * (n_params + n_outs),
                  out_specs=(PartitionSpec("core"),) * n_outs,
                  check_rep=False),
        donate_argnums=donate, keep_unused=True)

    def run(in_maps):
        per_core = [[np.asarray(m[name]) for name in in_names] for m in in_maps]
        concat_in = [np.concatenate([per_core[c][i] for c in range(B)], axis=0)
                     for i in range(n_params)]
        big_zeros = [np.concatenate([z] * B, axis=0) for z in zero_outs]
        outs = jax.block_until_ready(sharded(*concat_in, *big_zeros))
        results = []
        for c in range(B):
            r = {}
            for i, name in enumerate(out_names):
                rows = out_avals[i].shape[0]
                r[name] = np.asarray(outs[i][c * rows : (c + 1) * rows])
            results.append(r)
        return results

    _CACHE["runner"] = run
    return run


def kernel(**inputs) -> np.ndarray:
    in_maps, x_shape = _prep_inputs(**inputs)
    run = _get_runner()
    results = run(in_maps)
    out = np.stack([results[b]["y"].reshape(x_shape[1], x_shape[2], 1)
                    for b in range(B)])
    return out.astype(np.float32)



# revision 11
# speedup vs baseline: 2.8318x; 1.0540x over previous
"""Trainium2 Bass kernel for the GNO (Galerkin-type linear attention) model.

Reference computation per batch element b (N=4096 tokens, d=64):
    h = x @ lift_w + lift_b
    for each of 4 layers:
        q = h@q_w+q_b ; k = h@k_w+k_b ; v = h@v_w+v_b
        kern     = (q @ k^T) / sqrt(d)          # [N, N], no softmax!
        integral = (kern @ v) / N               # [N, d]
        h        = gelu(h@blk_w+blk_b + integral)
    out = h @ proj_w + proj_b

Because there is no softmax, (q k^T) v == q (k^T v), so each layer only
needs the tiny [64,64] moment matrix ktv = k^T v.  Further,
    integral = q @ (s*ktv)  = H_aug^T (q_w_aug @ (s*ktv))
    wh + integral           = H_aug^T (blk_w_aug + q_w_aug @ (s*ktv))
with H_aug = [h^T; 1] (a ones row folds every bias into the weights), so
the whole layer update is ONE [65,64] combined weight matmul + gelu.

Sharding: batch is 2 -> data-parallel on 2 NeuronCores, one batch element
per core, everything resident in SBUF.  Sequence-sharding wider would need
a per-layer AllReduce of ktv whose ~10us latency floor exceeds the whole
per-layer compute, so it loses.
"""

import os
import sys

for _p in ("/opt/trn_rl_repo", "/root/.axon_site/_ro/trn_rl_repo"):
    if os.path.isdir(_p) and _p not in sys.path:
        sys.path.append(_p)

import numpy as np

N = 4096          # tokens per batch element (64*64)
D = 64            # hidden
DA = D + 1        # hidden + ones row
L = 4             # layers
B = 2             # batch / cores used
SCALE = (1.0 / np.sqrt(np.float32(D))) / np.float32(N)

_CACHE = {}


def _build_nc():
    """Build + compile the per-core Bass program (identical on both cores).

    All large matmuls run in fp16 (1 PE cycle/row vs fp32's 4): the PE is
    row-throughput-bound here, so this is a ~4x tensor-time cut.  PSUM
    accumulation stays fp32, and the tiny ktv -> W_upd chain stays fp32,
    so the only precision loss is fp16 rounding (~5e-4) of H / weights.
    """
    import concourse.bass as bass
    import concourse.tile as tile
    from concourse import bacc, mybir

    f32 = mybir.dt.float32
    f16 = mybir.dt.float16
    ts = bass.ts
    GELU = mybir.ActivationFunctionType.Gelu

    nc = bacc.Bacc("TRN2", target_bir_lowering=False, debug=False, num_devices=B)

    xt_d = nc.dram_tensor("xt", [4, N], f16, kind="ExternalInput")
    lift_d = nc.dram_tensor("liftw", [4, DA], f16, kind="ExternalInput")
    kvw_d = nc.dram_tensor("kvw", [DA, L * 2 * D], f16, kind="ExternalInput")
    qts_d = nc.dram_tensor("qts", [D, L * DA], f16, kind="ExternalInput")
    blkw_d = nc.dram_tensor("blkw", [DA, L * D], f32, kind="ExternalInput")
    proj_d = nc.dram_tensor("projw", [DA, 1], f16, kind="ExternalInput")
    y_d = nc.dram_tensor("y", [1, N], f32, kind="ExternalOutput")

    PS = bass.MemorySpace.PSUM

    with tile.TileContext(nc) as tc:
        with (
            tc.tile_pool(name="consts", bufs=1) as consts,
            tc.tile_pool(name="hbuf", bufs=1) as hbuf,
            tc.tile_pool(name="kvsb", bufs=9) as kvsb,
            tc.tile_pool(name="small", bufs=2) as small,
            tc.tile_pool(name="ps_kv", bufs=3, space=PS) as ps_kv,
            tc.tile_pool(name="ps_sm", bufs=1, space=PS) as ps_sm,
            tc.tile_pool(name="ps_up", bufs=2, space=PS) as ps_up,
        ):
            # ---- load everything into SBUF -------------------------------
            xt = consts.tile([4, N], f16, tag="xt")
            nc.sync.dma_start(xt[:], xt_d.ap())
            liftw = consts.tile([4, DA], f16, tag="liftw")
            nc.sync.dma_start(liftw[:], lift_d.ap())
            kvw = consts.tile([DA, L * 2 * D], f16, tag="kvw")
            nc.sync.dma_start(kvw[:], kvw_d.ap())
            qts = consts.tile([D, L * DA], f16, tag="qts")
            nc.sync.dma_start(qts[:], qts_d.ap())
            blkw = consts.tile([DA, L * D], f32, tag="blkw")
            nc.sync.dma_start(blkw[:], blkw_d.ap())
            projw = consts.tile([DA, 1], f16, tag="projw")
            nc.sync.dma_start(projw[:], proj_d.ap())

            # two ping-pong H_aug buffers, [65, 4096] each
            H0 = hbuf.tile([DA, N], f16, tag="h0")
            H1 = hbuf.tile([DA, N], f16, tag="h1")
            # H1's ones row never gets written by the layer loop; seed it
            # from the ones row of x^T_aug.  H0's comes from the lift matmul.
            nc.sync.dma_start(H1[D : D + 1, :], xt_d.ap()[3:4, :])

            # ---- lift: H0 = lift_aug^T @ xt  ([65, 4096]) ----------------
            for c in range(4):
                ps = ps_up.tile([DA, 1024], f32, tag="up")
                for i in range(2):
                    nc.tensor.matmul(ps[:, ts(i, 512)], liftw[:],
                                     xt[:, 1024 * c + 512 * i :
                                            1024 * c + 512 * (i + 1)],
                                     start=True, stop=True)
                if c % 2 == 0:
                    nc.vector.tensor_copy(H0[:, ts(c, 1024)], ps[:])
                else:
                    nc.scalar.copy(H0[:, ts(c, 1024)], ps[:])

            # ---- layers --------------------------------------------------
            # Per-layer phases (keeps the PE in long uninterrupted runs):
            #   1. all 32 KV matmuls; PSUM->fp16 casts split DVE/ACT
            #   2. all 32 ktv matmuls into two half-accumulators, with the
            #      W_eff matmul accumulated from both halves so only the
            #      second half's copy sits on the critical path
            #   3. 8 update matmuls + gelu
            ACT_CAST = (2, 5, 7)  # kv groups whose cast runs on the scalar eng
            for l in range(L):
                cur = H0 if l % 2 == 0 else H1
                nxt = H1 if l % 2 == 0 else H0
                kvw_l = kvw[:, l * 2 * D : (l + 1) * 2 * D]

                # one PSUM bank subdivided: ktv halves + W_eff accumulator
                sm = ps_sm.tile([128, 512], f32, tag="sm")
                ktv_half = (sm[0:D, 0:64], sm[0:D, 64:128])
                weff = sm[0:DA, 128 : 128 + D]

                kv_tiles = []
                for j in range(8):
                    kv_ps = ps_kv.tile([128, 512], f32, tag="kv")
                    for k in range(4):
                        t = 4 * j + k
                        # KV_nat tile [128, 128] = H_chunk^T @ kvw_aug
                        nc.tensor.matmul(kv_ps[:, ts(k, 128)],
                                         cur[:, ts(t, 128)], kvw_l,
                                         start=True, stop=True)
                    kv_sb = kvsb.tile([128, 512], f16, tag="kvt")
                    if j in ACT_CAST:
                        nc.scalar.copy(kv_sb[:], kv_ps[:])
                    else:
                        nc.vector.tensor_copy(kv_sb[:], kv_ps[:])
                    kv_tiles.append(kv_sb)

                ktv_sbs = []
                for h in range(2):
                    for j in range(4 * h, 4 * h + 4):
                        kv_sb = kv_tiles[j]
                        for k in range(4):
                            first = (j % 4 == 0 and k == 0)
                            last = (j % 4 == 3 and k == 3)
                            # ktv_half += K_tile^T @ V_tile
                            nc.tensor.matmul(
                                ktv_half[h],
                                kv_sb[:, k * 128 : k * 128 + 64],
                                kv_sb[:, k * 128 + 64 : k * 128 + 128],
                                start=first, stop=last)
                    ktv_sb = small.tile([D, D], f16, tag=f"ktv{h}")
                    nc.vector.tensor_copy(ktv_sb[:], ktv_half[h])
                    ktv_sbs.append(ktv_sb)

                # W_eff = q_w_aug*s @ (ktv_a + ktv_b), accumulated in PSUM
                qts_l = qts[:, l * DA : (l + 1) * DA]
                nc.tensor.matmul(weff, qts_l, ktv_sbs[0][:],
                                 start=True, stop=False)
                nc.tensor.matmul(weff, qts_l, ktv_sbs[1][:],
                                 start=False, stop=True)
                wupd_sb = small.tile([DA, D], f16, tag="wupd")
                nc.vector.tensor_add(wupd_sb[:], weff,
                                     blkw[:, l * D : (l + 1) * D])

                # h' = gelu(H_aug^T @ W_upd), written transposed into nxt
                for c in range(4):
                    up_ps = ps_up.tile([D, 1024], f32, tag="up")
                    for i in range(2):
                        nc.tensor.matmul(
                            up_ps[:, ts(i, 512)], wupd_sb[:],
                            cur[:, 1024 * c + 512 * i : 1024 * c + 512 * (i + 1)],
                            start=True, stop=True)
                    nc.scalar.activation(nxt[0:D, ts(c, 1024)], up_ps[:], GELU)

            # ---- proj: y = proj_aug^T @ H_final  ([1, 4096]) -------------
            Hf = H0 if L % 2 == 0 else H1
            out_sb = consts.tile([1, N], f32, tag="out")
            for c in range(8):
                pr_ps = ps_kv.tile([1, 512], f32, tag="kv")
                nc.tensor.matmul(pr_ps[:], projw[:], Hf[:, ts(c, 512)],
                                 start=True, stop=True)
                nc.vector.tensor_copy(out_sb[0:1, ts(c, 512)], pr_ps[:])
            nc.sync.dma_start(y_d.ap(), out_sb[:])

    nc.compile()
    return nc


def _prep_inputs(x, lift_w, lift_b, blk_w, blk_b, q_w, q_b, k_w, k_b, v_w,
                 v_b, proj_w, proj_b):
    """Host-side weight packing (tiny [64,64] reshuffles, negligible cost)."""
    f = lambda a: np.asarray(a, dtype=np.float32)
    x = f(x)
    lift_w, lift_b = f(lift_w), f(lift_b)
    blk_w, blk_b = f(blk_w), f(blk_b)
    q_w, q_b, k_w, k_b, v_w, v_b = f(q_w), f(q_b), f(k_w), f(k_b), f(v_w), f(v_b)
    proj_w, proj_b = f(proj_w), f(proj_b)

    lift_aug = np.zeros((4, DA), np.float32)
    lift_aug[:3, :D] = lift_w
    lift_aug[3, :D] = lift_b
    lift_aug[3, D] = 1.0  # makes the lift matmul emit H0's ones row

    kvw = np.concatenate(
        [np.concatenate([np.vstack([k_w[l], k_b[l][None]]),
                         np.vstack([v_w[l], v_b[l][None]])], axis=1)
         for l in range(L)], axis=1).astype(np.float16)          # [65, 512]
    qts = np.concatenate(
        [(np.vstack([q_w[l], q_b[l][None]]) * SCALE).T
         for l in range(L)], axis=1).astype(np.float16)          # [64, 260]
    blkw = np.concatenate(
        [np.vstack([blk_w[l], blk_b[l][None]]) for l in range(L)],
        axis=1).astype(np.float32)                               # [65, 256]
    proj = np.vstack([proj_w, proj_b[None]]).astype(np.float16)  # [65, 1]
    lift_aug = lift_aug.astype(np.float16)

    in_maps = []
    for b in range(B):
        xt = np.concatenate([x[b].reshape(N, 3).T,
                             np.ones((1, N), np.float32)], axis=0)
        in_maps.append({"xt": np.ascontiguousarray(xt.astype(np.float16)),
                        "liftw": lift_aug,
                        "kvw": kvw, "qts": qts, "blkw": blkw, "projw": proj})
    return in_maps, x.shape


def _get_runner():
    """Compile once, return a fn(in_maps) -> list[{name: np.ndarray}]."""
    if "runner" in _CACHE:
        return _CACHE["runner"]

    import jax
    from jax.sharding import Mesh, PartitionSpec
    try:
        from jax.experimental.shard_map import shard_map
    except ImportError:  # newer jax
        from jax.sharding import shard_map
    from concourse import mybir
    from concourse.bass2jax import (_bass_exec_p, install_neuronx_cc_hook,
                                    partition_id_tensor)

    nc = _build_nc()
    install_neuronx_cc_hook()

    partition_name = (nc.partition_id_tensor.name
                      if nc.partition_id_tensor else None)
    in_names, out_names, out_avals, zero_outs = [], [], [], []
    for alloc in nc.m.functions[0].allocations:
        if not isinstance(alloc, mybir.MemoryLocationSet):
            continue
        name = alloc.memorylocations[0].name
        if alloc.kind == "ExternalInput":
            if name != partition_name:
                in_names.append(name)
        elif alloc.kind == "ExternalOutput":
            shape = tuple(alloc.tensor_shape)
            dtype = mybir.dt.np(alloc.dtype)
            out_names.append(name)
            out_avals.append(jax.core.ShapedArray(shape, dtype))
            zero_outs.append(np.zeros(shape, dtype))
    n_params = len(in_names)
    n_outs = len(out_avals)
    all_in_names = in_names + out_names + ([partition_name] if partition_name else [])
    donate = tuple(range(n_params, n_params + n_outs))

    def _body(*args):
        operands = list(args)
        if partition_name is not None:
            operands.append(partition_id_tensor())
        return tuple(_bass_exec_p.bind(
            *operands, out_avals=tuple(out_avals), in_names=tuple(all_in_names),
            out_names=tuple(out_names), lowering_input_output_aliases=(),
            sim_require_finite=True, sim_require_nnan=True, nc=nc))

    devices = jax.devices()[:B]
    mesh = Mesh(np.asarray(devices), ("core",))
    sharded = jax.jit(
        shard_map(_body, mesh=mesh,
                  in_specs=(PartitionSpec("core"),) * (n_params + n_outs),
                  out_specs=(PartitionSpec("core"),) * n_outs,
                  check_rep=False),
        donate_argnums=donate, keep_unused=True)

    def run(in_maps):
        per_core = [[np.asarray(m[name]) for name in in_names] for m in in_maps]
        concat_in = [np.concatenate([per_core[c][i] for c in range(B)], axis=0)
                     for i in range(n_params)]
        big_zeros = [np.concatenate([z] * B, axis=0) for z in zero_outs]
        outs = jax.block_until_ready(sharded(*concat_in, *big_zeros))
        results = []
        for c in range(B):
            r = {}
            for i, name in enumerate(out_names):
                rows = out_avals[i].shape[0]
                r[name] = np.asarray(outs[i][c * rows : (c + 1) * rows])
            results.append(r)
        return results

    _CACHE["runner"] = run
    return run


def kernel(**inputs) -> np.ndarray:
    in_maps, x_shape = _prep_inputs(**inputs)
    run = _get_runner()
    results = run(in_maps)
    out = np.stack([results[b]["y"].reshape(x_shape[1], x_shape[2], 1)
                    for b in range(B)])
    return out.astype(np.float32)



# revision 13
# speedup vs baseline: 2.8425x; 1.0038x over previous
"""Trainium2 Bass kernel for the GNO (Galerkin-type linear attention) model.

Reference computation per batch element b (N=4096 tokens, d=64):
    h = x @ lift_w + lift_b
    for each of 4 layers:
        q = h@q_w+q_b ; k = h@k_w+k_b ; v = h@v_w+v_b
        kern     = (q @ k^T) / sqrt(d)          # [N, N], no softmax!
        integral = (kern @ v) / N               # [N, d]
        h        = gelu(h@blk_w+blk_b + integral)
    out = h @ proj_w + proj_b

Because there is no softmax, (q k^T) v == q (k^T v), so each layer only
needs the tiny [64,64] moment matrix ktv = k^T v.  Further,
    integral = q @ (s*ktv)  = H_aug^T (q_w_aug @ (s*ktv))
    wh + integral           = H_aug^T (blk_w_aug + q_w_aug @ (s*ktv))
with H_aug = [h^T; 1] (a ones row folds every bias into the weights), so
the whole layer update is ONE [65,64] combined weight matmul + gelu.

Sharding: batch is 2 -> data-parallel on 2 NeuronCores, one batch element
per core, everything resident in SBUF.  Sequence-sharding wider would need
a per-layer AllReduce of ktv whose ~10us latency floor exceeds the whole
per-layer compute, so it loses.
"""

import os
import sys

for _p in ("/opt/trn_rl_repo", "/root/.axon_site/_ro/trn_rl_repo"):
    if os.path.isdir(_p) and _p not in sys.path:
        sys.path.append(_p)

import numpy as np

try:
    from ml_dtypes import bfloat16 as ML_BF16
except ImportError:  # jax always bundles ml_dtypes
    import jax.numpy as _jnp
    ML_BF16 = _jnp.bfloat16

N = 4096          # tokens per batch element (64*64)
D = 64            # hidden
DA = D + 1        # hidden + ones row
L = 4             # layers
B = 2             # batch / cores used
SCALE = (1.0 / np.sqrt(np.float32(D))) / np.float32(N)

_CACHE = {}


def _build_nc():
    """Build + compile the per-core Bass program (identical on both cores).

    All large matmuls run in fp16 (1 PE cycle/row vs fp32's 4): the PE is
    row-throughput-bound here, so this is a ~4x tensor-time cut.  PSUM
    accumulation stays fp32, and the tiny ktv -> W_upd chain stays fp32,
    so the only precision loss is fp16 rounding (~5e-4) of H / weights.
    """
    import concourse.bass as bass
    import concourse.tile as tile
    from concourse import bacc, mybir

    f32 = mybir.dt.float32
    f16 = mybir.dt.bfloat16
    ts = bass.ts
    GELU = mybir.ActivationFunctionType.Gelu

    nc = bacc.Bacc("TRN2", target_bir_lowering=False, debug=False, num_devices=B)

    xt_d = nc.dram_tensor("xt", [4, N], f16, kind="ExternalInput")
    lift_d = nc.dram_tensor("liftw", [4, DA], f16, kind="ExternalInput")
    kvw_d = nc.dram_tensor("kvw", [DA, L * 2 * D], f16, kind="ExternalInput")
    qts_d = nc.dram_tensor("qts", [D, L * DA], f16, kind="ExternalInput")
    blkw_d = nc.dram_tensor("blkw", [DA, L * D], f32, kind="ExternalInput")
    proj_d = nc.dram_tensor("projw", [DA, 1], f16, kind="ExternalInput")
    y_d = nc.dram_tensor("y", [1, N], f32, kind="ExternalOutput")

    PS = bass.MemorySpace.PSUM

    with tile.TileContext(nc) as tc:
        with (
            tc.tile_pool(name="consts", bufs=1) as consts,
            tc.tile_pool(name="hbuf", bufs=1) as hbuf,
            tc.tile_pool(name="kvsb", bufs=9) as kvsb,
            tc.tile_pool(name="small", bufs=2) as small,
            tc.tile_pool(name="ps_kv", bufs=3, space=PS) as ps_kv,
            tc.tile_pool(name="ps_sm", bufs=1, space=PS) as ps_sm,
            tc.tile_pool(name="ps_up", bufs=2, space=PS) as ps_up,
        ):
            # ---- load everything into SBUF -------------------------------
            xt = consts.tile([4, N], f16, tag="xt")
            nc.sync.dma_start(xt[:], xt_d.ap())
            liftw = consts.tile([4, DA], f16, tag="liftw")
            nc.sync.dma_start(liftw[:], lift_d.ap())
            kvw = consts.tile([DA, L * 2 * D], f16, tag="kvw")
            nc.sync.dma_start(kvw[:], kvw_d.ap())
            qts = consts.tile([D, L * DA], f16, tag="qts")
            nc.sync.dma_start(qts[:], qts_d.ap())
            blkw = consts.tile([DA, L * D], f32, tag="blkw")
            nc.sync.dma_start(blkw[:], blkw_d.ap())
            projw = consts.tile([DA, 1], f16, tag="projw")
            nc.sync.dma_start(projw[:], proj_d.ap())

            # two ping-pong H_aug buffers, [65, 4096] each
            H0 = hbuf.tile([DA, N], f16, tag="h0")
            H1 = hbuf.tile([DA, N], f16, tag="h1")
            # H1's ones row never gets written by the layer loop; seed it
            # from the ones row of x^T_aug.  H0's comes from the lift matmul.
            nc.sync.dma_start(H1[D : D + 1, :], xt_d.ap()[3:4, :])

            # ---- lift: H0 = lift_aug^T @ xt  ([65, 4096]) ----------------
            for c in range(4):
                ps = ps_up.tile([DA, 1024], f32, tag="up")
                for i in range(2):
                    nc.tensor.matmul(ps[:, ts(i, 512)], liftw[:],
                                     xt[:, 1024 * c + 512 * i :
                                            1024 * c + 512 * (i + 1)],
                                     start=True, stop=True)
                if c % 2 == 0:
                    nc.vector.tensor_copy(H0[:, ts(c, 1024)], ps[:])
                else:
                    nc.scalar.copy(H0[:, ts(c, 1024)], ps[:])

            # ---- layers --------------------------------------------------
            # Per-layer phases (keeps the PE in long uninterrupted runs):
            #   1. all 32 KV matmuls; PSUM->fp16 casts split DVE/ACT
            #   2. all 32 ktv matmuls into two half-accumulators, with the
            #      W_eff matmul accumulated from both halves so only the
            #      second half's copy sits on the critical path
            #   3. 8 update matmuls + gelu
            ACT_CAST = (2, 5, 7)  # kv groups whose cast runs on the scalar eng
            for l in range(L):
                cur = H0 if l % 2 == 0 else H1
                nxt = H1 if l % 2 == 0 else H0
                kvw_l = kvw[:, l * 2 * D : (l + 1) * 2 * D]

                # one PSUM bank subdivided: ktv halves + W_eff accumulator
                sm = ps_sm.tile([128, 512], f32, tag="sm")
                ktv_half = (sm[0:D, 0:64], sm[0:D, 64:128])
                weff = sm[0:DA, 128 : 128 + D]

                kv_tiles = []
                for j in range(8):
                    kv_ps = ps_kv.tile([128, 512], f32, tag="kv")
                    for k in range(4):
                        t = 4 * j + k
                        # KV_nat tile [128, 128] = H_chunk^T @ kvw_aug
                        nc.tensor.matmul(kv_ps[:, ts(k, 128)],
                                         cur[:, ts(t, 128)], kvw_l,
                                         start=True, stop=True)
                    kv_sb = kvsb.tile([128, 512], f16, tag="kvt")
                    if j in ACT_CAST:
                        nc.scalar.copy(kv_sb[:], kv_ps[:])
                    else:
                        nc.vector.tensor_copy(kv_sb[:], kv_ps[:])
                    kv_tiles.append(kv_sb)

                ktv_sbs = []
                for h in range(2):
                    for j in range(4 * h, 4 * h + 4):
                        kv_sb = kv_tiles[j]
                        for k in range(4):
                            first = (j % 4 == 0 and k == 0)
                            last = (j % 4 == 3 and k == 3)
                            # ktv_half += K_tile^T @ V_tile
                            nc.tensor.matmul(
                                ktv_half[h],
                                kv_sb[:, k * 128 : k * 128 + 64],
                                kv_sb[:, k * 128 + 64 : k * 128 + 128],
                                start=first, stop=last)
                    ktv_sb = small.tile([D, D], f16, tag=f"ktv{h}")
                    nc.vector.tensor_copy(ktv_sb[:], ktv_half[h])
                    ktv_sbs.append(ktv_sb)

                # W_eff = q_w_aug*s @ (ktv_a + ktv_b), accumulated in PSUM
                qts_l = qts[:, l * DA : (l + 1) * DA]
                nc.tensor.matmul(weff, qts_l, ktv_sbs[0][:],
                                 start=True, stop=False)
                nc.tensor.matmul(weff, qts_l, ktv_sbs[1][:],
                                 start=False, stop=True)
                wupd_sb = small.tile([DA, D], f16, tag="wupd")
                nc.vector.tensor_add(wupd_sb[:], weff,
                                     blkw[:, l * D : (l + 1) * D])

                # h' = gelu(H_aug^T @ W_upd), written transposed into nxt
                for c in range(4):
                    up_ps = ps_up.tile([D, 1024], f32, tag="up")
                    for i in range(2):
                        nc.tensor.matmul(
                            up_ps[:, ts(i, 512)], wupd_sb[:],
                            cur[:, 1024 * c + 512 * i : 1024 * c + 512 * (i + 1)],
                            start=True, stop=True)
                    nc.scalar.activation(nxt[0:D, ts(c, 1024)], up_ps[:], GELU)

            # ---- proj: y = proj_aug^T @ H_final  ([1, 4096]) -------------
            Hf = H0 if L % 2 == 0 else H1
            out_sb = consts.tile([1, N], f32, tag="out")
            for c in range(8):
                pr_ps = ps_kv.tile([1, 512], f32, tag="kv")
                nc.tensor.matmul(pr_ps[:], projw[:], Hf[:, ts(c, 512)],
                                 start=True, stop=True)
                nc.vector.tensor_copy(out_sb[0:1, ts(c, 512)], pr_ps[:])
            nc.sync.dma_start(y_d.ap(), out_sb[:])

    nc.compile()
    return nc


def _prep_inputs(x, lift_w, lift_b, blk_w, blk_b, q_w, q_b, k_w, k_b, v_w,
                 v_b, proj_w, proj_b):
    """Host-side weight packing (tiny [64,64] reshuffles, negligible cost)."""
    f = lambda a: np.asarray(a, dtype=np.float32)
    x = f(x)
    lift_w, lift_b = f(lift_w), f(lift_b)
    blk_w, blk_b = f(blk_w), f(blk_b)
    q_w, q_b, k_w, k_b, v_w, v_b = f(q_w), f(q_b), f(k_w), f(k_b), f(v_w), f(v_b)
    proj_w, proj_b = f(proj_w), f(proj_b)

    lift_aug = np.zeros((4, DA), np.float32)
    lift_aug[:3, :D] = lift_w
    lift_aug[3, :D] = lift_b
    lift_aug[3, D] = 1.0  # makes the lift matmul emit H0's ones row

    kvw = np.concatenate(
        [np.concatenate([np.vstack([k_w[l], k_b[l][None]]),
                         np.vstack([v_w[l], v_b[l][None]])], axis=1)
         for l in range(L)], axis=1).astype(ML_BF16)          # [65, 512]
    qts = np.concatenate(
        [(np.vstack([q_w[l], q_b[l][None]]) * SCALE).T
         for l in range(L)], axis=1).astype(ML_BF16)          # [64, 260]
    blkw = np.concatenate(
        [np.vstack([blk_w[l], blk_b[l][None]]) for l in range(L)],
        axis=1).astype(np.float32)                               # [65, 256]
    proj = np.vstack([proj_w, proj_b[None]]).astype(ML_BF16)  # [65, 1]
    lift_aug = lift_aug.astype(ML_BF16)

    in_maps = []
    for b in range(B):
        xt = np.concatenate([x[b].reshape(N, 3).T,
                             np.ones((1, N), np.float32)], axis=0)
        in_maps.append({"xt": np.ascontiguousarray(xt.astype(ML_BF16)),
                        "liftw": lift_aug,
                        "kvw": kvw, "qts": qts, "blkw": blkw, "projw": proj})
    return in_maps, x.shape


def _get_runner():
    """Compile once, return a fn(in_maps) -> list[{name: np.ndarray}]."""
    if "runner" in _CACHE:
        return _CACHE["runner"]

    import jax
    from jax.sharding import Mesh, PartitionSpec
    try:
        from jax.experimental.shard_map import shard_map
    except ImportError:  # newer jax
        from jax.sharding import shard_map
    from concourse import mybir
    from concourse.bass2jax import (_bass_exec_p, install_neuronx_cc_hook,
                                    partition_id_tensor)

    nc = _build_nc()
    install_neuronx_cc_hook()

    partition_name = (nc.partition_id_tensor.name
                      if nc.partition_id_tensor else None)
    in_names, out_names, out_avals, zero_outs = [], [], [], []
    for alloc in nc.m.functions[0].allocations:
        if not isinstance(alloc, mybir.MemoryLocationSet):
            continue
        name = alloc.memorylocations[0].name
        if alloc.kind == "ExternalInput":
            if name != partition_name:
                in_names.append(name)
        elif alloc.kind == "ExternalOutput":
            shape = tuple(alloc.tensor_shape)
            dtype = mybir.dt.np(alloc.dtype)
            out_names.append(name)
            out_avals.append(jax.core.ShapedArray(shape, dtype))
            zero_outs.append(np.zeros(shape, dtype))
    n_params = len(in_names)
    n_outs = len(out_avals)
    all_in_names = in_names + out_names + ([partition_name] if partition_name else [])
    donate = tuple(range(n_params, n_params + n_outs))

    def _body(*args):
        operands = list(args)
        if partition_name is not None:
            operands.append(partition_id_tensor())
        return tuple(_bass_exec_p.bind(
            *operands, out_avals=tuple(out_avals), in_names=tuple(all_in_names),
            out_names=tuple(out_names), lowering_input_output_aliases=(),
            sim_require_finite=True, sim_require_nnan=True, nc=nc))

    devices = jax.devices()[:B]
    mesh = Mesh(np.asarray(devices), ("core",))
    sharded = jax.jit(
        shard_map(_body, mesh=mesh,
                  in_specs=(PartitionSpec("core"),) * (n_params + n_outs),
                  out_specs=(PartitionSpec("core"),) * n_outs,
                  check_rep=False),
        donate_argnums=donate, keep_unused=True)

    def run(in_maps):
        per_core = [[np.asarray(m[name]) for name in in_names] for m in in_maps]
        concat_in = [np.concatenate([per_core[c][i] for c in range(B)], axis=0)
                     for i in range(n_params)]
        big_zeros = [np.concatenate([z] * B, axis=0) for z in zero_outs]
        outs = jax.block_until_ready(sharded(*concat_in, *big_zeros))
        results = []
        for c in range(B):
            r = {}
            for i, name in enumerate(out_names):
                rows = out_avals[i].shape[0]
                r[name] = np.asarray(outs[i][c * rows : (c + 1) * rows])
            results.append(r)
        return results

    _CACHE["runner"] = run
    return run


def kernel(**inputs) -> np.ndarray:
    in_maps, x_shape = _prep_inputs(**inputs)
    run = _get_runner()
    results = run(in_maps)
    out = np.stack([results[b]["y"].reshape(x_shape[1], x_shape[2], 1)
                    for b in range(B)])
    return out.astype(np.float32)



# revision 15
# speedup vs baseline: 2.9096x; 1.0236x over previous
"""Trainium2 Bass kernel for the GNO (Galerkin-type linear attention) model.

Reference computation per batch element b (N=4096 tokens, d=64):
    h = x @ lift_w + lift_b
    for each of 4 layers:
        q = h@q_w+q_b ; k = h@k_w+k_b ; v = h@v_w+v_b
        kern     = (q @ k^T) / sqrt(d)          # [N, N], no softmax!
        integral = (kern @ v) / N               # [N, d]
        h        = gelu(h@blk_w+blk_b + integral)
    out = h @ proj_w + proj_b

Because there is no softmax, (q k^T) v == q (k^T v), so each layer only
needs the tiny [64,64] moment matrix ktv = k^T v.  Further,
    integral = q @ (s*ktv)  = H_aug^T (q_w_aug @ (s*ktv))
    wh + integral           = H_aug^T (blk_w_aug + q_w_aug @ (s*ktv))
with H_aug = [h^T; 1] (a ones row folds every bias into the weights), so
the whole layer update is ONE [65,64] combined weight matmul + gelu.

Sharding: batch is 2 -> data-parallel on 2 NeuronCores, one batch element
per core, everything resident in SBUF.  Sequence-sharding wider would need
a per-layer AllReduce of ktv whose ~10us latency floor exceeds the whole
per-layer compute, so it loses.
"""

import os
import sys

for _p in ("/opt/trn_rl_repo", "/root/.axon_site/_ro/trn_rl_repo"):
    if os.path.isdir(_p) and _p not in sys.path:
        sys.path.append(_p)

import numpy as np

try:
    from ml_dtypes import bfloat16 as ML_BF16
except ImportError:  # jax always bundles ml_dtypes
    import jax.numpy as _jnp
    ML_BF16 = _jnp.bfloat16

N = 4096          # tokens per batch element (64*64)
D = 64            # hidden
DA = D + 1        # hidden + ones row
L = 4             # layers
B = 2             # batch / cores used
SCALE = (1.0 / np.sqrt(np.float32(D))) / np.float32(N)

_CACHE = {}


def _build_nc():
    """Build + compile the per-core Bass program (identical on both cores).

    All large matmuls run in fp16 (1 PE cycle/row vs fp32's 4): the PE is
    row-throughput-bound here, so this is a ~4x tensor-time cut.  PSUM
    accumulation stays fp32, and the tiny ktv -> W_upd chain stays fp32,
    so the only precision loss is fp16 rounding (~5e-4) of H / weights.
    """
    import concourse.bass as bass
    import concourse.tile as tile
    from concourse import bacc, mybir

    f32 = mybir.dt.float32
    f16 = mybir.dt.float16
    ts = bass.ts
    GELU = mybir.ActivationFunctionType.Gelu

    nc = bacc.Bacc("TRN2", target_bir_lowering=False, debug=False, num_devices=B)

    xt_d = nc.dram_tensor("xt", [4, N], f16, kind="ExternalInput")
    lift_d = nc.dram_tensor("liftw", [4, DA], f16, kind="ExternalInput")
    kvw_d = nc.dram_tensor("kvw", [DA, L * 2 * D], f16, kind="ExternalInput")
    qts_d = nc.dram_tensor("qts", [D, L * DA], f16, kind="ExternalInput")
    blkw_d = nc.dram_tensor("blkw", [DA, L * D], f32, kind="ExternalInput")
    proj_d = nc.dram_tensor("projw", [DA, 1], f16, kind="ExternalInput")
    y_d = nc.dram_tensor("y", [1, N], f32, kind="ExternalOutput")

    PS = bass.MemorySpace.PSUM

    with tile.TileContext(nc) as tc:
        with (
            tc.tile_pool(name="consts", bufs=1) as consts,
            tc.tile_pool(name="hbuf", bufs=1) as hbuf,
            tc.tile_pool(name="kvsb", bufs=9) as kvsb,
            tc.tile_pool(name="small", bufs=2) as small,
            tc.tile_pool(name="ps_kv", bufs=3, space=PS) as ps_kv,
            tc.tile_pool(name="ps_sm", bufs=1, space=PS) as ps_sm,
            tc.tile_pool(name="ps_up", bufs=2, space=PS) as ps_up,
        ):
            # ---- load everything into SBUF -------------------------------
            # liftw + per-chunk xt DMAs go first (they gate the first
            # matmuls); the rest is spread over other trigger engines.
            liftw = consts.tile([4, DA], f16, tag="liftw")
            nc.sync.dma_start(liftw[:], lift_d.ap())
            xt = consts.tile([4, N], f16, tag="xt")
            for c in range(4):
                nc.sync.dma_start(xt[:, ts(c, 1024)],
                                  xt_d.ap()[:, ts(c, 1024)])
            kvw = consts.tile([DA, L * 2 * D], f16, tag="kvw")
            nc.gpsimd.dma_start(kvw[:], kvw_d.ap())
            qts = consts.tile([D, L * DA], f16, tag="qts")
            nc.gpsimd.dma_start(qts[:], qts_d.ap())
            blkw = consts.tile([DA, L * D], f32, tag="blkw")
            nc.gpsimd.dma_start(blkw[:], blkw_d.ap())
            projw = consts.tile([DA, 1], f16, tag="projw")
            nc.gpsimd.dma_start(projw[:], proj_d.ap())

            # two ping-pong H_aug buffers, [65, 4096] each
            H0 = hbuf.tile([DA, N], f16, tag="h0")
            H1 = hbuf.tile([DA, N], f16, tag="h1")
            # H1's ones row never gets written by the layer loop; seed it
            # from the ones row of x^T_aug.  H0's comes from the lift matmul.
            nc.scalar.dma_start(H1[D : D + 1, :], xt_d.ap()[3:4, :])

            # ---- emission helpers ---------------------------------------
            ACT_CAST = (1, 4, 6)  # kv groups whose cast runs on the scalar eng

            def H(l):
                return H0 if l % 2 == 0 else H1

            def emit_lift_chunk(c):
                # H0[:, chunk c] = lift_aug^T @ xt chunk  (1024 cols)
                ps = ps_up.tile([DA, 1024], f32, tag="up")
                for i in range(2):
                    nc.tensor.matmul(ps[:, ts(i, 512)], liftw[:],
                                     xt[:, 1024 * c + 512 * i :
                                            1024 * c + 512 * (i + 1)],
                                     start=True, stop=True)
                if c % 2 == 0:
                    nc.vector.tensor_copy(H0[:, ts(c, 1024)], ps[:])
                else:
                    nc.scalar.copy(H0[:, ts(c, 1024)], ps[:])

            def emit_kv_group(l, j, kv_tiles):
                kvw_l = kvw[:, l * 2 * D : (l + 1) * 2 * D]
                kv_ps = ps_kv.tile([128, 512], f32, tag="kv")
                for k in range(4):
                    t = 4 * j + k
                    # KV_nat tile [128, 128] = H_chunk^T @ kvw_aug
                    nc.tensor.matmul(kv_ps[:, ts(k, 128)],
                                     H(l)[:, ts(t, 128)], kvw_l,
                                     start=True, stop=True)
                kv_sb = kvsb.tile([128, 512], f16, tag="kvt")
                if j in ACT_CAST:
                    nc.scalar.copy(kv_sb[:], kv_ps[:])
                else:
                    nc.vector.tensor_copy(kv_sb[:], kv_ps[:])
                kv_tiles.append(kv_sb)

            def emit_ktv_and_wupd(l, kv_tiles):
                # one PSUM bank subdivided: ktv halves + W_eff accumulator
                sm = ps_sm.tile([128, 512], f32, tag="sm")
                ktv_half = (sm[0:D, 0:64], sm[0:D, 64:128])
                weff = sm[0:DA, 128 : 128 + D]
                ktv_sbs = []
                for h in range(2):
                    for j in range(4 * h, 4 * h + 4):
                        kv_sb = kv_tiles[j]
                        for k in range(4):
                            first = (j % 4 == 0 and k == 0)
                            last = (j % 4 == 3 and k == 3)
                            # ktv_half += K_tile^T @ V_tile
                            nc.tensor.matmul(
                                ktv_half[h],
                                kv_sb[:, k * 128 : k * 128 + 64],
                                kv_sb[:, k * 128 + 64 : k * 128 + 128],
                                start=first, stop=last)
                    ktv_sb = small.tile([D, D], f16, tag=f"ktv{h}")
                    nc.vector.tensor_copy(ktv_sb[:], ktv_half[h])
                    ktv_sbs.append(ktv_sb)
                # W_eff = q_w_aug*s @ (ktv_a + ktv_b), accumulated in PSUM
                qts_l = qts[:, l * DA : (l + 1) * DA]
                nc.tensor.matmul(weff, qts_l, ktv_sbs[0][:],
                                 start=True, stop=False)
                nc.tensor.matmul(weff, qts_l, ktv_sbs[1][:],
                                 start=False, stop=True)
                wupd_sb = small.tile([DA, D], f16, tag="wupd")
                nc.vector.tensor_add(wupd_sb[:], weff,
                                     blkw[:, l * D : (l + 1) * D])
                return wupd_sb

            def emit_upd_chunk(l, wupd_sb, c):
                # nxt[:, chunk c] = gelu(H_aug^T @ W_upd)  (1024 cols)
                up_ps = ps_up.tile([D, 1024], f32, tag="up")
                for i in range(2):
                    nc.tensor.matmul(
                        up_ps[:, ts(i, 512)], wupd_sb[:],
                        H(l)[:, 1024 * c + 512 * i : 1024 * c + 512 * (i + 1)],
                        start=True, stop=True)
                nc.scalar.activation(H(l + 1)[0:D, ts(c, 1024)], up_ps[:],
                                     GELU)

            out_sb = consts.tile([1, N], f32, tag="out")

            def emit_proj_chunk(c):
                pr_ps = ps_kv.tile([1, 512], f32, tag="kv")
                nc.tensor.matmul(pr_ps[:], projw[:],
                                 H(L)[:, ts(c, 512)], start=True, stop=True)
                nc.vector.tensor_copy(out_sb[0:1, ts(c, 512)], pr_ps[:])

            # ---- pipelined emission -------------------------------------
            # Each layer l: its KV groups interleave with the producer of
            # its H (lift for l=0, layer l-1's update otherwise), then the
            # ktv phase + W_upd chain.  The last layer's update interleaves
            # with the proj matmuls.
            wupd_prev = None
            for l in range(L):
                kv_tiles = []

                def prod(i, l=l):
                    if l == 0:
                        emit_lift_chunk(i)
                    else:
                        emit_upd_chunk(l - 1, wupd_prev, i)

                prod(0)
                prod(1)
                emit_kv_group(l, 0, kv_tiles)
                emit_kv_group(l, 1, kv_tiles)
                prod(2)
                emit_kv_group(l, 2, kv_tiles)
                emit_kv_group(l, 3, kv_tiles)
                prod(3)
                for j in range(4, 8):
                    emit_kv_group(l, j, kv_tiles)
                wupd_prev = emit_ktv_and_wupd(l, kv_tiles)

            # final update + proj, interleaved
            emit_upd_chunk(L - 1, wupd_prev, 0)
            emit_upd_chunk(L - 1, wupd_prev, 1)
            emit_proj_chunk(0)
            emit_proj_chunk(1)
            emit_upd_chunk(L - 1, wupd_prev, 2)
            emit_proj_chunk(2)
            emit_proj_chunk(3)
            emit_upd_chunk(L - 1, wupd_prev, 3)
            for c in range(4, 8):
                emit_proj_chunk(c)
                if c == 5:
                    nc.sync.dma_start(y_d.ap()[:, 0:2048], out_sb[:, 0:2048])
            nc.sync.dma_start(y_d.ap()[:, 2048:N], out_sb[:, 2048:N])

    nc.compile()
    return nc


def _prep_inputs(x, lift_w, lift_b, blk_w, blk_b, q_w, q_b, k_w, k_b, v_w,
                 v_b, proj_w, proj_b):
    """Host-side weight packing (tiny [64,64] reshuffles, negligible cost)."""
    f = lambda a: np.asarray(a, dtype=np.float32)
    x = f(x)
    lift_w, lift_b = f(lift_w), f(lift_b)
    blk_w, blk_b = f(blk_w), f(blk_b)
    q_w, q_b, k_w, k_b, v_w, v_b = f(q_w), f(q_b), f(k_w), f(k_b), f(v_w), f(v_b)
    proj_w, proj_b = f(proj_w), f(proj_b)

    lift_aug = np.zeros((4, DA), np.float32)
    lift_aug[:3, :D] = lift_w
    lift_aug[3, :D] = lift_b
    lift_aug[3, D] = 1.0  # makes the lift matmul emit H0's ones row

    kvw = np.concatenate(
        [np.concatenate([np.vstack([k_w[l], k_b[l][None]]),
                         np.vstack([v_w[l], v_b[l][None]])], axis=1)
         for l in range(L)], axis=1).astype(np.float16)          # [65, 512]
    qts = np.concatenate(
        [(np.vstack([q_w[l], q_b[l][None]]) * SCALE).T
         for l in range(L)], axis=1).astype(np.float16)          # [64, 260]
    blkw = np.concatenate(
        [np.vstack([blk_w[l], blk_b[l][None]]) for l in range(L)],
        axis=1).astype(np.float32)                               # [65, 256]
    proj = np.vstack([proj_w, proj_b[None]]).astype(np.float16)  # [65, 1]
    lift_aug = lift_aug.astype(np.float16)

    in_maps = []
    for b in range(B):
        xt = np.concatenate([x[b].reshape(N, 3).T,
                             np.ones((1, N), np.float32)], axis=0)
        in_maps.append({"xt": np.ascontiguousarray(xt.astype(np.float16)),
                        "liftw": lift_aug,
                        "kvw": kvw, "qts": qts, "blkw": blkw, "projw": proj})
    return in_maps, x.shape


def _get_runner():
    """Compile once, return a fn(in_maps) -> list[{name: np.ndarray}]."""
    if "runner" in _CACHE:
        return _CACHE["runner"]

    import jax
    from jax.sharding import Mesh, PartitionSpec
    try:
        from jax.experimental.shard_map import shard_map
    except ImportError:  # newer jax
        from jax.sharding import shard_map
    from concourse import mybir
    from concourse.bass2jax import (_bass_exec_p, install_neuronx_cc_hook,
                                    partition_id_tensor)

    nc = _build_nc()
    install_neuronx_cc_hook()

    partition_name = (nc.partition_id_tensor.name
                      if nc.partition_id_tensor else None)
    in_names, out_names, out_avals, zero_outs = [], [], [], []
    for alloc in nc.m.functions[0].allocations:
        if not isinstance(alloc, mybir.MemoryLocationSet):
            continue
        name = alloc.memorylocations[0].name
        if alloc.kind == "ExternalInput":
            if name != partition_name:
                in_names.append(name)
        elif alloc.kind == "ExternalOutput":
            shape = tuple(alloc.tensor_shape)
            dtype = mybir.dt.np(alloc.dtype)
            out_names.append(name)
            out_avals.append(jax.core.ShapedArray(shape, dtype))
            zero_outs.append(np.zeros(shape, dtype))
    n_params = len(in_names)
    n_outs = len(out_avals)
    all_in_names = in_names + out_names + ([partition_name] if partition_name else [])
    donate = tuple(range(n_params, n_params + n_outs))

    def _body(*args):
        operands = list(args)
        if partition_name is not None:
            operands.append(partition_id_tensor())
        return tuple(_bass_exec_p.bind(
            *operands, out_avals=tuple(out_avals), in_names=tuple(all_in_names),
            out_names=tuple(out_names), lowering_input_output_aliases=(),
            sim_require_finite=True, sim_require_nnan=True, nc=nc))

    devices = jax.devices()[:B]
    mesh = Mesh(np.asarray(devices), ("core",))
    sharded = jax.jit(
        shard_map(_body, mesh=mesh,
                  in_specs=(PartitionSpec("core"),) * (n_params + n_outs),
                  out_specs=(PartitionSpec("core"),) * n_outs,
                  check_rep=False),
        donate_argnums=donate, keep_unused=True)

    def run(in_maps):
        per_core = [[np.asarray(m[name]) for name in in_names] for m in in_maps]
        concat_in = [np.concatenate([per_core[c][i] for c in range(B)], axis=0)
                     for i in range(n_params)]
        big_zeros = [np.concatenate([z] * B, axis=0) for z in zero_outs]
        outs = jax.block_until_ready(sharded(*concat_in, *big_zeros))
        results = []
        for c in range(B):
            r = {}
            for i, name in enumerate(out_names):
                rows = out_avals[i].shape[0]
                r[name] = np.asarray(outs[i][c * rows : (c + 1) * rows])
            results.append(r)
        return results

    _CACHE["runner"] = run
    return run


def kernel(**inputs) -> np.ndarray:
    in_maps, x_shape = _prep_inputs(**inputs)
    run = _get_runner()
    results = run(in_maps)
    out = np.stack([results[b]["y"].reshape(x_shape[1], x_shape[2], 1)
                    for b in range(B)])
    return out.astype(np.float32)

